# revision 59
# baseline (speedup 1.0000x reference)
"""Locally-connected 2D layer on 8 Trainium2 NeuronCores.

Problem: x[128,3,64,64] f32, per-position weights W[60,60,32,75], bias b[60,60,32]
  out[b,o,y,x] = sum_k patches[b,y,x,k] * W[y,x,o,k] + b[y,x,o],  k=(c,dy,dx)

Strategy (spatial sharding over output rows, 8 rows/core, memory-regime):
  - Groups of 4 consecutive x positions share one full-array matmul: the
    contraction is the UNION of the 4 patch windows, planes (c, dy, dx') with
    dx' in 0..7 -> 5*3*8 = 120 partitions.  The stationary [120, 128] holds
    all 4 positions' weights ((j,o) columns, structural zeros where dx'-j is
    outside 0..4), so each [128, 128] matmul output is fully useful:
    out[(j,o), b] for 4 x-positions at once.  15 matmuls per row, 120 per
    core, N=128 streaming.  Bias is added on the host after gathering.
  - dy is handled with a mod-5 ring of row-slots (24 planes each); the per-row
    dy rotation is folded into the host-side W slab layout (np.roll), so the
    device always reads xp[0:120] as one fixed partition range.
  - All input HBM traffic is a few wide DMAs: initial ring fill [120, 3840B]
    on the gpsimd SWDGE queue (the fast path, ~300GB/s) together with the W
    slab in four [128, 7680B] row-pair DMAs; the future-row slabs ride the
    HWDGE queues.  Ring advances are SBUF->SBUF copies from the staged future
    slabs, chunk-gated in two halves per row on the sync queue.
  - Output is bf16 (host upcasts): per-row [128, 3840B] stores, rows 0-4 and
    7 (split in two) on gpsimd, rows 5-6 on the HWDGE queues.  PSUM
    evacuation alternates vector/scalar engines.
  - Dummy matmuls on a scratch tile run during the initial DMA fill to keep
    the PE busy.
"""

import numpy as np

B, C, H, WIDTH = 128, 3, 64, 64
KH = KW = 5
RY = RX = 60
O = 32
NCORES = 8
RPC = 8             # output rows computed per core (8*8=64, last 4 dropped)
INR = RPC + KH - 1  # 12 input rows per core
PADH = NCORES * RPC + KH - 1  # 68
NG = 15             # groups of 4 x-positions per row
NPL = C * 8         # 24 planes per ring slot (c, dx' in 0..7)
KP = KH * NPL       # 120 contraction partitions
FU = NG * B         # 1920 free elems per plane (g, b)
CHUNKS = ((0, 4), (4, 4), (8, 4), (12, 3))  # (first group, n groups) per PSUM chunk
NWARM = 24

_cache = {}


def _build():
    import concourse.bass as bass
    import concourse.bacc as bacc
    import concourse.tile as tile
    import concourse.mybir as mybir

    f32 = mybir.dt.float32
    din = mybir.dt.bfloat16
    nc = bacc.Bacc("TRN2", target_bir_lowering=False, debug=False,
                   num_devices=NCORES)
    ui_d = nc.dram_tensor("ui", [KP, FU], din, kind="ExternalInput")
    ufa_d = nc.dram_tensor("ufa", [KP, FU], din, kind="ExternalInput")
    ufb_d = nc.dram_tensor("ufb", [2 * NPL, FU], din, kind="ExternalInput")
    w_d = nc.dram_tensor("w", [128, RPC * NG * B], din, kind="ExternalInput")
    oc_d = nc.dram_tensor("oc", [RPC, 4, O, NG, B], din, kind="ExternalOutput")

    with tile.TileContext(nc) as tc:
        with (
            tc.tile_pool(name="const", bufs=1) as cpool,
            tc.tile_pool(name="os", bufs=3) as opool,
            tc.tile_pool(name="ps", bufs=4, space=bass.MemorySpace.PSUM) as ppool,
            tc.tile_pool(name="pw", bufs=1, space=bass.MemorySpace.PSUM) as wpool,
        ):
            xp = cpool.tile([KP, FU], din)            # ring planes
            ufa = cpool.tile([KP, FU], din)           # future rows 5-9
            ufb = cpool.tile([2 * NPL, FU], din)      # future rows 10-11
            ws = cpool.tile([128, RPC * NG * B], din)
            dm = cpool.tile([1, 512], din)            # warmup operand

            nc.gpsimd.dma_start(xp[:], ui_d[:])
            for p in range(3):  # W row-pairs (0,1) (2,3) (4,5) on SWDGE
                c0, c1 = p * 2 * NG * B, (p + 1) * 2 * NG * B
                nc.gpsimd.dma_start(ws[:, c0:c1], w_d[:, c0:c1])
            nc.scalar.dma_start(ufa[:], ufa_d[:])
            # last W pair (rows 6,7) rides the idle-early scalar HWDGE queue:
            # it lands ~8us earlier than at the tail of the gpsimd stream,
            # so its (hoisted) wait no longer stalls early rows, and the
            # gpsimd queue's stores start sooner
            nc.scalar.dma_start(ws[:, 6 * NG * B:8 * NG * B],
                                w_d[:, 6 * NG * B:8 * NG * B])
            nc.sync.dma_start(ufb[:], ufb_d[:])

            # PE warmup: keep the array busy during the fill so HAM
            # un-throttles the PE clock before the first real matmul
            nc.vector.memset(dm[:], 1.0)
            pw = wpool.tile([1, 512], f32)
            for _ in range(NWARM):
                nc.tensor.matmul(pw[0:1, :], dm[:, 0:1], dm[:, :])

            for k in range(RPC):
                ot = opool.tile([128, FU], din)
                oc_k = oc_d[k].rearrange("j o g b -> (j o) (g b)")
                st_eng = nc.gpsimd if k < 5 else (nc.sync, nc.scalar, nc.sync)[k - 5]
                for ci, (g0, gn) in enumerate(CHUNKS):
                    pt = ppool.tile([128, 4 * B], f32)
                    for gg in range(gn):
                        g = g0 + gg
                        nc.tensor.matmul(
                            pt[:, gg * B:(gg + 1) * B],
                            ws[0:KP, (k * NG + g) * B:(k * NG + g + 1) * B],
                            xp[:, g * B:(g + 1) * B],
                        )
                    if ci % 2 == 0:
                        nc.vector.tensor_copy(
                            ot[:, g0 * B:(g0 + gn) * B], pt[:, :gn * B])
                    else:
                        nc.scalar.copy(
                            ot[:, g0 * B:(g0 + gn) * B], pt[:, :gn * B])
                        f0, f1 = (0, 8 * B) if ci == 1 else (8 * B, FU)
                        if k + KH < INR:
                            # ring advance: slot k%5 <- input row k+5, copied
                            # SBUF->SBUF from the staged future slab
                            s = k % KH
                            src = (ufa[s * NPL:(s + 1) * NPL, f0:f1]
                                   if k + KH < 2 * KH else
                                   ufb[(k - KH) * NPL:(k - KH + 1) * NPL, f0:f1])
                            nc.sync.dma_start(
                                xp[s * NPL:(s + 1) * NPL, f0:f1], src)
                        st_eng.dma_start(oc_k[:, f0:f1], ot[:, f0:f1])

    nc.compile()
    return nc


def _get_nc():
    if "nc" not in _cache:
        _cache["nc"] = _build()
    return _cache["nc"]


def _prep_inputs(x, W, b):
    import ml_dtypes
    bf = ml_dtypes.bfloat16
    x = np.asarray(x, np.float32)
    W = np.asarray(W, np.float32)
    xh = np.zeros((PADH, C, WIDTH, B), np.float32)
    xh[:H] = x.transpose(2, 1, 3, 0)  # [row, c, w, batch]
    # union planes: U[row, (c,dx'), (g,b)] = xh[row, c, 4g+dx', b], dx' in 0..7
    U = np.zeros((PADH, C, 8, NG, B), np.float32)
    for dxp in range(8):
        U[:, :, dxp] = xh[:, :, dxp::4][:, :, :NG]
    U = U.reshape(PADH, NPL, FU).astype(bf)

    W5 = W.reshape(RY, RX, O, C, KH, KW)
    in_maps = []
    for i in range(NCORES):
        nk = min(RPC, RY - RPC * i)
        W5c = np.zeros((RPC, NG, 4, O, C, KH, KW), np.float32)
        W5c[:nk] = W5[RPC * i:RPC * i + nk].reshape(nk, NG, 4, O, C, KH, KW)
        A = W5c.transpose(5, 4, 6, 0, 1, 2, 3)  # [dy, c, dx, k, g, j, o]
        D = np.zeros((KH, C, 8, RPC, NG, 4, O), np.float32)
        for j in range(4):
            D[:, :, j:j + KW, :, :, j, :] = A[:, :, :, :, :, j, :]
        S = np.empty_like(D)  # slot rm holds dy=(rm-k)%5 -> roll dy by k
        for k in range(RPC):
            S[:, :, :, k] = np.roll(D[:, :, :, k], k, axis=0)
        wslab = np.zeros((128, RPC * NG * 4 * O), np.float32)
        wslab[:KP] = S.reshape(KP, -1)

        Uc = U[RPC * i:RPC * i + INR]  # [12, 24, FU]
        in_maps.append({
            "ui": np.ascontiguousarray(Uc[:KH].reshape(KP, FU)),
            "ufa": np.ascontiguousarray(Uc[KH:2 * KH].reshape(KP, FU)),
            "ufb": np.ascontiguousarray(Uc[2 * KH:].reshape(2 * NPL, FU)),
            "w": wslab.astype(bf),
        })
    return in_maps


def kernel(x, W, b):
    from concourse.bass_utils import run_bass_kernel_spmd

    nc = _get_nc()
    in_maps = _prep_inputs(x, W, b)
    br = run_bass_kernel_spmd(nc, in_maps, list(range(NCORES)),
                              **_cache.get("run_kwargs", {}))
    _cache["last_run"] = br
    oc = np.stack([np.asarray(br.results[i]["oc"]) for i in range(NCORES)])
    oc = oc.reshape(NCORES * RPC, 4, O, NG, B).astype(np.float32)
    out = oc.transpose(4, 2, 0, 3, 1).reshape(B, O, NCORES * RPC, RX)
    out = out[:, :, :RY, :] + np.asarray(b, np.float32).transpose(2, 0, 1)[None]
    return np.ascontiguousarray(out)


# revision 61
# speedup vs baseline: 1.0493x; 1.0493x over previous
"""Locally-connected 2D layer on 8 Trainium2 NeuronCores.

Problem: x[128,3,64,64] f32, per-position weights W[60,60,32,75], bias b[60,60,32]
  out[b,o,y,x] = sum_k patches[b,y,x,k] * W[y,x,o,k] + b[y,x,o],  k=(c,dy,dx)

Strategy (spatial sharding over output rows, 8 rows/core, memory-regime):
  - Groups of 4 consecutive x positions share one full-array matmul: the
    contraction is the UNION of the 4 patch windows, planes (c, dy, dx') with
    dx' in 0..7 -> 5*3*8 = 120 partitions.  The stationary [120, 128] holds
    all 4 positions' weights ((j,o) columns, structural zeros where dx'-j is
    outside 0..4), so each [128, 128] matmul output is fully useful:
    out[(j,o), b] for 4 x-positions at once.  15 matmuls per row, 120 per
    core, N=128 streaming.  Bias is added on the host after gathering.
  - dy is handled with a mod-5 ring of row-slots (24 planes each); the per-row
    dy rotation is folded into the host-side W slab layout (np.roll), so the
    device always reads xp[0:120] as one fixed partition range.
  - All input HBM traffic is a few wide DMAs: initial ring fill [120, 3840B]
    on the gpsimd SWDGE queue (the fast path, ~300GB/s) together with the W
    slab in four [128, 7680B] row-pair DMAs; the future-row slabs ride the
    HWDGE queues.  Ring advances are SBUF->SBUF copies from the staged future
    slabs, chunk-gated in two halves per row on the sync queue.
  - Output is bf16 (host upcasts): per-row [128, 3840B] stores, rows 0-4 and
    7 (split in two) on gpsimd, rows 5-6 on the HWDGE queues.  PSUM
    evacuation alternates vector/scalar engines.
  - Dummy matmuls on a scratch tile run during the initial DMA fill to keep
    the PE busy.
"""

import numpy as np

B, C, H, WIDTH = 128, 3, 64, 64
KH = KW = 5
RY = RX = 60
O = 32
NCORES = 8
RPC = 8             # output rows computed per core (8*8=64, last 4 dropped)
INR = RPC + KH - 1  # 12 input rows per core
PADH = NCORES * RPC + KH - 1  # 68
NG = 15             # groups of 4 x-positions per row
NPL = C * 8         # 24 planes per ring slot (c, dx' in 0..7)
KP = KH * NPL       # 120 contraction partitions
FU = NG * B         # 1920 free elems per plane (g, b)
CHUNKS = ((0, 4), (4, 4), (8, 4), (12, 3))  # (first group, n groups) per PSUM chunk
NWARM = 0

_cache = {}


def _build():
    import concourse.bass as bass
    import concourse.bacc as bacc
    import concourse.tile as tile
    import concourse.mybir as mybir

    f32 = mybir.dt.float32
    din = mybir.dt.bfloat16
    nc = bacc.Bacc("TRN2", target_bir_lowering=False, debug=False,
                   num_devices=NCORES)
    ui_d = nc.dram_tensor("ui", [KP, FU], din, kind="ExternalInput")
    ufa_d = nc.dram_tensor("ufa", [KP, FU], din, kind="ExternalInput")
    ufb_d = nc.dram_tensor("ufb", [2 * NPL, FU], din, kind="ExternalInput")
    w_d = nc.dram_tensor("w", [128, RPC * NG * B], din, kind="ExternalInput")
    oc_d = nc.dram_tensor("oc", [RPC, 4, O, NG, B], din, kind="ExternalOutput")

    with tile.TileContext(nc) as tc:
        with (
            tc.tile_pool(name="const", bufs=1) as cpool,
            tc.tile_pool(name="os", bufs=3) as opool,
            tc.tile_pool(name="ps", bufs=4, space=bass.MemorySpace.PSUM) as ppool,
            tc.tile_pool(name="pw", bufs=1, space=bass.MemorySpace.PSUM) as wpool,
        ):
            xp = cpool.tile([KP, FU], din)            # ring planes
            ufa = cpool.tile([KP, FU], din)           # future rows 5-9
            ufb = cpool.tile([2 * NPL, FU], din)      # future rows 10-11
            ws = cpool.tile([128, RPC * NG * B], din)
            dm = cpool.tile([1, 512], din)            # warmup operand

            nc.gpsimd.dma_start(xp[:], ui_d[:])
            for p in range(4):  # W row-pairs on the gpsimd SWDGE queue
                c0, c1 = p * 2 * NG * B, (p + 1) * 2 * NG * B
                nc.gpsimd.dma_start(ws[:, c0:c1], w_d[:, c0:c1])
            nc.scalar.dma_start(ufa[:], ufa_d[:])
            nc.sync.dma_start(ufb[:], ufb_d[:])

            # PE warmup: keep the array busy during the fill so HAM
            # un-throttles the PE clock before the first real matmul
            nc.vector.memset(dm[:], 1.0)
            pw = wpool.tile([1, 512], f32)
            for _ in range(NWARM):
                nc.tensor.matmul(pw[0:1, :], dm[:, 0:1], dm[:, :])

            for k in range(RPC):
                ot = opool.tile([128, FU], din)
                oc_k = oc_d[k].rearrange("j o g b -> (j o) (g b)")
                st_eng = nc.gpsimd if k < 5 else (nc.sync, nc.scalar, nc.sync)[k - 5]
                for ci, (g0, gn) in enumerate(CHUNKS):
                    pt = ppool.tile([128, 4 * B], f32)
                    for gg in range(gn):
                        g = g0 + gg
                        nc.tensor.matmul(
                            pt[:, gg * B:(gg + 1) * B],
                            ws[0:KP, (k * NG + g) * B:(k * NG + g + 1) * B],
                            xp[:, g * B:(g + 1) * B],
                        )
                    if ci % 2 == 0:
                        nc.vector.tensor_copy(
                            ot[:, g0 * B:(g0 + gn) * B], pt[:, :gn * B])
                    else:
                        nc.scalar.copy(
                            ot[:, g0 * B:(g0 + gn) * B], pt[:, :gn * B])
                        f0, f1 = (0, 8 * B) if ci == 1 else (8 * B, FU)
                        if k + KH < INR:
                            # ring advance: slot k%5 <- input row k+5, copied
                            # SBUF->SBUF from the staged future slab
                            s = k % KH
                            src = (ufa[s * NPL:(s + 1) * NPL, f0:f1]
                                   if k + KH < 2 * KH else
                                   ufb[(k - KH) * NPL:(k - KH + 1) * NPL, f0:f1])
                            nc.sync.dma_start(
                                xp[s * NPL:(s + 1) * NPL, f0:f1], src)
                        st_eng.dma_start(oc_k[:, f0:f1], ot[:, f0:f1])

    nc.compile()
    return nc


def _get_nc():
    if "nc" not in _cache:
        _cache["nc"] = _build()
    return _cache["nc"]


def _prep_inputs(x, W, b):
    import ml_dtypes
    bf = ml_dtypes.bfloat16
    x = np.asarray(x, np.float32)
    W = np.asarray(W, np.float32)
    xh = np.zeros((PADH, C, WIDTH, B), np.float32)
    xh[:H] = x.transpose(2, 1, 3, 0)  # [row, c, w, batch]
    # union planes: U[row, (c,dx'), (g,b)] = xh[row, c, 4g+dx', b], dx' in 0..7
    U = np.zeros((PADH, C, 8, NG, B), np.float32)
    for dxp in range(8):
        U[:, :, dxp] = xh[:, :, dxp::4][:, :, :NG]
    U = U.reshape(PADH, NPL, FU).astype(bf)

    W5 = W.reshape(RY, RX, O, C, KH, KW)
    in_maps = []
    for i in range(NCORES):
        nk = min(RPC, RY - RPC * i)
        W5c = np.zeros((RPC, NG, 4, O, C, KH, KW), np.float32)
        W5c[:nk] = W5[RPC * i:RPC * i + nk].reshape(nk, NG, 4, O, C, KH, KW)
        A = W5c.transpose(5, 4, 6, 0, 1, 2, 3)  # [dy, c, dx, k, g, j, o]
        D = np.zeros((KH, C, 8, RPC, NG, 4, O), np.float32)
        for j in range(4):
            D[:, :, j:j + KW, :, :, j, :] = A[:, :, :, :, :, j, :]
        S = np.empty_like(D)  # slot rm holds dy=(rm-k)%5 -> roll dy by k
        for k in range(RPC):
            S[:, :, :, k] = np.roll(D[:, :, :, k], k, axis=0)
        wslab = np.zeros((128, RPC * NG * 4 * O), np.float32)
        wslab[:KP] = S.reshape(KP, -1)

        Uc = U[RPC * i:RPC * i + INR]  # [12, 24, FU]
        in_maps.append({
            "ui": np.ascontiguousarray(Uc[:KH].reshape(KP, FU)),
            "ufa": np.ascontiguousarray(Uc[KH:2 * KH].reshape(KP, FU)),
            "ufb": np.ascontiguousarray(Uc[2 * KH:].reshape(2 * NPL, FU)),
            "w": wslab.astype(bf),
        })
    return in_maps


def kernel(x, W, b):
    from concourse.bass_utils import run_bass_kernel_spmd

    nc = _get_nc()
    in_maps = _prep_inputs(x, W, b)
    br = run_bass_kernel_spmd(nc, in_maps, list(range(NCORES)),
                              **_cache.get("run_kwargs", {}))
    _cache["last_run"] = br
    oc = np.stack([np.asarray(br.results[i]["oc"]) for i in range(NCORES)])
    oc = oc.reshape(NCORES * RPC, 4, O, NG, B).astype(np.float32)
    out = oc.transpose(4, 2, 0, 3, 1).reshape(B, O, NCORES * RPC, RX)
    out = out[:, :, :RY, :] + np.asarray(b, np.float32).transpose(2, 0, 1)[None]
    return np.ascontiguousarray(out)


# revision 64
# speedup vs baseline: 1.0539x; 1.0045x over previous
"""Locally-connected 2D layer on 8 Trainium2 NeuronCores.

Problem: x[128,3,64,64] f32, per-position weights W[60,60,32,75], bias b[60,60,32]
  out[b,o,y,x] = sum_k patches[b,y,x,k] * W[y,x,o,k] + b[y,x,o],  k=(c,dy,dx)

Strategy (spatial sharding over output rows, 8 rows/core, memory-regime):
  - Groups of 4 consecutive x positions share one full-array matmul: the
    contraction is the UNION of the 4 patch windows, planes (c, dy, dx') with
    dx' in 0..7 -> 5*3*8 = 120 partitions.  The stationary [120, 128] holds
    all 4 positions' weights ((j,o) columns, structural zeros where dx'-j is
    outside 0..4), so each [128, 128] matmul output is fully useful:
    out[(j,o), b] for 4 x-positions at once.  15 matmuls per row, 120 per
    core, N=128 streaming.  Bias is added on the host after gathering.
  - dy is handled with a mod-5 ring of row-slots (24 planes each); the per-row
    dy rotation is folded into the host-side W slab layout (np.roll), so the
    device always reads xp[0:120] as one fixed partition range.
  - All input HBM traffic is a few wide DMAs: initial ring fill [120, 3840B]
    on the gpsimd SWDGE queue (the fast path, ~300GB/s) together with the W
    slab in four [128, 7680B] row-pair DMAs; the future-row slabs ride the
    HWDGE queues.  Ring advances are SBUF->SBUF copies from the staged future
    slabs, chunk-gated in two halves per row on the sync queue.
  - Output is bf16 (host upcasts): per-row [128, 3840B] stores, rows 0-4 and
    7 (split in two) on gpsimd, rows 5-6 on the HWDGE queues.  PSUM
    evacuation alternates vector/scalar engines.
  - Dummy matmuls on a scratch tile run during the initial DMA fill to keep
    the PE busy.
"""

import numpy as np

B, C, H, WIDTH = 128, 3, 64, 64
KH = KW = 5
RY = RX = 60
O = 32
NCORES = 8
RPC = 8             # output rows computed per core (8*8=64, last 4 dropped)
INR = RPC + KH - 1  # 12 input rows per core
PADH = NCORES * RPC + KH - 1  # 68
NG = 15             # groups of 4 x-positions per row
NPL = C * 8         # 24 planes per ring slot (c, dx' in 0..7)
KP = KH * NPL       # 120 contraction partitions
FU = NG * B         # 1920 free elems per plane (g, b)
CHUNKS = ((0, 4), (4, 4), (8, 4), (12, 3))  # (first group, n groups) per PSUM chunk
NWARM = 0

_cache = {}


def _build():
    import concourse.bass as bass
    import concourse.bacc as bacc
    import concourse.tile as tile
    import concourse.mybir as mybir

    f32 = mybir.dt.float32
    din = mybir.dt.bfloat16
    nc = bacc.Bacc("TRN2", target_bir_lowering=False, debug=False,
                   num_devices=NCORES)
    ui_d = nc.dram_tensor("ui", [KP, FU], din, kind="ExternalInput")
    ufa_d = nc.dram_tensor("ufa", [KP, FU], din, kind="ExternalInput")
    ufb_d = nc.dram_tensor("ufb", [2 * NPL, FU], din, kind="ExternalInput")
    w_d = nc.dram_tensor("w", [128, RPC * NG * B], din, kind="ExternalInput")
    oc_d = nc.dram_tensor("oc", [RPC, 4, O, NG, B], din, kind="ExternalOutput")

    with tile.TileContext(nc) as tc:
        with (
            tc.tile_pool(name="const", bufs=1) as cpool,
            tc.tile_pool(name="os", bufs=3) as opool,
            tc.tile_pool(name="ps", bufs=5, space=bass.MemorySpace.PSUM) as ppool,
        ):
            xp = cpool.tile([KP, FU], din)            # ring planes
            ufa = cpool.tile([KP, FU], din)           # future rows 5-9
            ufb = cpool.tile([2 * NPL, FU], din)      # future rows 10-11
            ws = cpool.tile([128, RPC * NG * B], din)

            nc.gpsimd.dma_start(xp[:], ui_d[:])
            for p in range(4):  # W row-pairs on the gpsimd SWDGE queue
                c0, c1 = p * 2 * NG * B, (p + 1) * 2 * NG * B
                nc.gpsimd.dma_start(ws[:, c0:c1], w_d[:, c0:c1])
            nc.scalar.dma_start(ufa[:], ufa_d[:])
            nc.sync.dma_start(ufb[:], ufb_d[:])

            for k in range(RPC):
                ot = opool.tile([128, FU], din)
                oc_k = oc_d[k].rearrange("j o g b -> (j o) (g b)")
                st_eng = nc.gpsimd if k < 5 else (nc.sync, nc.scalar, nc.sync)[k - 5]
                for ci, (g0, gn) in enumerate(CHUNKS):
                    pt = ppool.tile([128, 4 * B], f32)
                    for gg in range(gn):
                        g = g0 + gg
                        nc.tensor.matmul(
                            pt[:, gg * B:(gg + 1) * B],
                            ws[0:KP, (k * NG + g) * B:(k * NG + g + 1) * B],
                            xp[:, g * B:(g + 1) * B],
                        )
                    if ci % 2 == 0:
                        nc.vector.tensor_copy(
                            ot[:, g0 * B:(g0 + gn) * B], pt[:, :gn * B])
                    else:
                        nc.scalar.copy(
                            ot[:, g0 * B:(g0 + gn) * B], pt[:, :gn * B])
                        f0, f1 = (0, 8 * B) if ci == 1 else (8 * B, FU)
                        if k + KH < INR:
                            # ring advance: slot k%5 <- input row k+5, copied
                            # SBUF->SBUF from the staged future slab
                            s = k % KH
                            src = (ufa[s * NPL:(s + 1) * NPL, f0:f1]
                                   if k + KH < 2 * KH else
                                   ufb[(k - KH) * NPL:(k - KH + 1) * NPL, f0:f1])
                            nc.sync.dma_start(
                                xp[s * NPL:(s + 1) * NPL, f0:f1], src)
                        st_eng.dma_start(oc_k[:, f0:f1], ot[:, f0:f1])

    nc.compile()
    return nc


def _get_nc():
    if "nc" not in _cache:
        _cache["nc"] = _build()
    return _cache["nc"]


def _prep_inputs(x, W, b):
    import ml_dtypes
    bf = ml_dtypes.bfloat16
    x = np.asarray(x, np.float32)
    W = np.asarray(W, np.float32)
    xh = np.zeros((PADH, C, WIDTH, B), np.float32)
    xh[:H] = x.transpose(2, 1, 3, 0)  # [row, c, w, batch]
    # union planes: U[row, (c,dx'), (g,b)] = xh[row, c, 4g+dx', b], dx' in 0..7
    U = np.zeros((PADH, C, 8, NG, B), np.float32)
    for dxp in range(8):
        U[:, :, dxp] = xh[:, :, dxp::4][:, :, :NG]
    U = U.reshape(PADH, NPL, FU).astype(bf)

    W5 = W.reshape(RY, RX, O, C, KH, KW)
    in_maps = []
    for i in range(NCORES):
        nk = min(RPC, RY - RPC * i)
        W5c = np.zeros((RPC, NG, 4, O, C, KH, KW), np.float32)
        W5c[:nk] = W5[RPC * i:RPC * i + nk].reshape(nk, NG, 4, O, C, KH, KW)
        A = W5c.transpose(5, 4, 6, 0, 1, 2, 3)  # [dy, c, dx, k, g, j, o]
        D = np.zeros((KH, C, 8, RPC, NG, 4, O), np.float32)
        for j in range(4):
            D[:, :, j:j + KW, :, :, j, :] = A[:, :, :, :, :, j, :]
        S = np.empty_like(D)  # slot rm holds dy=(rm-k)%5 -> roll dy by k
        for k in range(RPC):
            S[:, :, :, k] = np.roll(D[:, :, :, k], k, axis=0)
        wslab = np.zeros((128, RPC * NG * 4 * O), np.float32)
        wslab[:KP] = S.reshape(KP, -1)

        Uc = U[RPC * i:RPC * i + INR]  # [12, 24, FU]
        in_maps.append({
            "ui": np.ascontiguousarray(Uc[:KH].reshape(KP, FU)),
            "ufa": np.ascontiguousarray(Uc[KH:2 * KH].reshape(KP, FU)),
            "ufb": np.ascontiguousarray(Uc[2 * KH:].reshape(2 * NPL, FU)),
            "w": wslab.astype(bf),
        })
    return in_maps


def kernel(x, W, b):
    from concourse.bass_utils import run_bass_kernel_spmd

    nc = _get_nc()
    in_maps = _prep_inputs(x, W, b)
    br = run_bass_kernel_spmd(nc, in_maps, list(range(NCORES)),
                              **_cache.get("run_kwargs", {}))
    _cache["last_run"] = br
    oc = np.stack([np.asarray(br.results[i]["oc"]) for i in range(NCORES)])
    oc = oc.reshape(NCORES * RPC, 4, O, NG, B).astype(np.float32)
    out = oc.transpose(4, 2, 0, 3, 1).reshape(B, O, NCORES * RPC, RX)
    out = out[:, :, :RY, :] + np.asarray(b, np.float32).transpose(2, 0, 1)[None]
    return np.ascontiguousarray(out)


# revision 65
# speedup vs baseline: 1.0572x; 1.0031x over previous
"""Locally-connected 2D layer on 8 Trainium2 NeuronCores.

Problem: x[128,3,64,64] f32, per-position weights W[60,60,32,75], bias b[60,60,32]
  out[b,o,y,x] = sum_k patches[b,y,x,k] * W[y,x,o,k] + b[y,x,o],  k=(c,dy,dx)

Strategy (spatial sharding over output rows, 8 rows/core, memory-regime):
  - Groups of 4 consecutive x positions share one full-array matmul: the
    contraction is the UNION of the 4 patch windows, planes (c, dy, dx') with
    dx' in 0..7 -> 5*3*8 = 120 partitions.  The stationary [120, 128] holds
    all 4 positions' weights ((j,o) columns, structural zeros where dx'-j is
    outside 0..4), so each [128, 128] matmul output is fully useful:
    out[(j,o), b] for 4 x-positions at once.  15 matmuls per row, 120 per
    core, N=128 streaming.  Bias is added on the host after gathering.
  - dy is handled with a mod-5 ring of row-slots (24 planes each); the per-row
    dy rotation is folded into the host-side W slab layout (np.roll), so the
    device always reads xp[0:120] as one fixed partition range.
  - All input HBM traffic is a few wide DMAs: initial ring fill [120, 3840B]
    on the gpsimd SWDGE queue (the fast path, ~300GB/s) together with the W
    slab in four [128, 7680B] row-pair DMAs; the future-row slabs ride the
    HWDGE queues.  Ring advances are SBUF->SBUF copies from the staged future
    slabs, chunk-gated in two halves per row on the sync queue.
  - Output is bf16 (host upcasts): per-row [128, 3840B] stores, rows 0-4 and
    7 (split in two) on gpsimd, rows 5-6 on the HWDGE queues.  PSUM
    evacuation alternates vector/scalar engines.
  - Dummy matmuls on a scratch tile run during the initial DMA fill to keep
    the PE busy.
"""

import numpy as np

B, C, H, WIDTH = 128, 3, 64, 64
KH = KW = 5
RY = RX = 60
O = 32
NCORES = 8
RPC = 8             # output rows computed per core (8*8=64, last 4 dropped)
INR = RPC + KH - 1  # 12 input rows per core
PADH = NCORES * RPC + KH - 1  # 68
NG = 15             # groups of 4 x-positions per row
NPL = C * 8         # 24 planes per ring slot (c, dx' in 0..7)
KP = KH * NPL       # 120 contraction partitions
FU = NG * B         # 1920 free elems per plane (g, b)
CHUNKS = ((0, 4), (4, 4), (8, 4), (12, 3))  # (first group, n groups) per PSUM chunk
NWARM = 0

_cache = {}


def _build():
    import concourse.bass as bass
    import concourse.bacc as bacc
    import concourse.tile as tile
    import concourse.mybir as mybir

    f32 = mybir.dt.float32
    din = mybir.dt.bfloat16
    nc = bacc.Bacc("TRN2", target_bir_lowering=False, debug=False,
                   num_devices=NCORES)
    ui_d = nc.dram_tensor("ui", [KP, FU], din, kind="ExternalInput")
    ufa_d = nc.dram_tensor("ufa", [KP, FU], din, kind="ExternalInput")
    ufb_d = nc.dram_tensor("ufb", [2 * NPL, FU], din, kind="ExternalInput")
    w_d = nc.dram_tensor("w", [128, RPC * NG * B], din, kind="ExternalInput")
    oc_d = nc.dram_tensor("oc", [RPC, 4, O, NG, B], din, kind="ExternalOutput")

    with tile.TileContext(nc) as tc:
        with (
            tc.tile_pool(name="const", bufs=1) as cpool,
            tc.tile_pool(name="os", bufs=3) as opool,
            tc.tile_pool(name="ps", bufs=6, space=bass.MemorySpace.PSUM) as ppool,
        ):
            xp = cpool.tile([KP, FU], din)            # ring planes
            ufa = cpool.tile([KP, FU], din)           # future rows 5-9
            ufb = cpool.tile([2 * NPL, FU], din)      # future rows 10-11
            ws = cpool.tile([128, RPC * NG * B], din)

            nc.gpsimd.dma_start(xp[:], ui_d[:])
            for p in range(4):  # W row-pairs on the gpsimd SWDGE queue
                c0, c1 = p * 2 * NG * B, (p + 1) * 2 * NG * B
                nc.gpsimd.dma_start(ws[:, c0:c1], w_d[:, c0:c1])
            nc.scalar.dma_start(ufa[:], ufa_d[:])
            nc.sync.dma_start(ufb[:], ufb_d[:])

            for k in range(RPC):
                ot = opool.tile([128, FU], din)
                oc_k = oc_d[k].rearrange("j o g b -> (j o) (g b)")
                st_eng = nc.gpsimd if k < 5 else (nc.sync, nc.scalar, nc.sync)[k - 5]
                for ci, (g0, gn) in enumerate(CHUNKS):
                    pt = ppool.tile([128, 4 * B], f32)
                    for gg in range(gn):
                        g = g0 + gg
                        nc.tensor.matmul(
                            pt[:, gg * B:(gg + 1) * B],
                            ws[0:KP, (k * NG + g) * B:(k * NG + g + 1) * B],
                            xp[:, g * B:(g + 1) * B],
                        )
                    if ci % 2 == 0:
                        nc.vector.tensor_copy(
                            ot[:, g0 * B:(g0 + gn) * B], pt[:, :gn * B])
                    else:
                        nc.scalar.copy(
                            ot[:, g0 * B:(g0 + gn) * B], pt[:, :gn * B])
                        f0, f1 = (0, 8 * B) if ci == 1 else (8 * B, FU)
                        if k + KH < INR:
                            # ring advance: slot k%5 <- input row k+5, copied
                            # SBUF->SBUF from the staged future slab
                            s = k % KH
                            src = (ufa[s * NPL:(s + 1) * NPL, f0:f1]
                                   if k + KH < 2 * KH else
                                   ufb[(k - KH) * NPL:(k - KH + 1) * NPL, f0:f1])
                            nc.sync.dma_start(
                                xp[s * NPL:(s + 1) * NPL, f0:f1], src)
                        st_eng.dma_start(oc_k[:, f0:f1], ot[:, f0:f1])

    nc.compile()
    return nc


def _get_nc():
    if "nc" not in _cache:
        _cache["nc"] = _build()
    return _cache["nc"]


def _prep_inputs(x, W, b):
    import ml_dtypes
    bf = ml_dtypes.bfloat16
    x = np.asarray(x, np.float32)
    W = np.asarray(W, np.float32)
    xh = np.zeros((PADH, C, WIDTH, B), np.float32)
    xh[:H] = x.transpose(2, 1, 3, 0)  # [row, c, w, batch]
    # union planes: U[row, (c,dx'), (g,b)] = xh[row, c, 4g+dx', b], dx' in 0..7
    U = np.zeros((PADH, C, 8, NG, B), np.float32)
    for dxp in range(8):
        U[:, :, dxp] = xh[:, :, dxp::4][:, :, :NG]
    U = U.reshape(PADH, NPL, FU).astype(bf)

    W5 = W.reshape(RY, RX, O, C, KH, KW)
    in_maps = []
    for i in range(NCORES):
        nk = min(RPC, RY - RPC * i)
        W5c = np.zeros((RPC, NG, 4, O, C, KH, KW), np.float32)
        W5c[:nk] = W5[RPC * i:RPC * i + nk].reshape(nk, NG, 4, O, C, KH, KW)
        A = W5c.transpose(5, 4, 6, 0, 1, 2, 3)  # [dy, c, dx, k, g, j, o]
        D = np.zeros((KH, C, 8, RPC, NG, 4, O), np.float32)
        for j in range(4):
            D[:, :, j:j + KW, :, :, j, :] = A[:, :, :, :, :, j, :]
        S = np.empty_like(D)  # slot rm holds dy=(rm-k)%5 -> roll dy by k
        for k in range(RPC):
            S[:, :, :, k] = np.roll(D[:, :, :, k], k, axis=0)
        wslab = np.zeros((128, RPC * NG * 4 * O), np.float32)
        wslab[:KP] = S.reshape(KP, -1)

        Uc = U[RPC * i:RPC * i + INR]  # [12, 24, FU]
        in_maps.append({
            "ui": np.ascontiguousarray(Uc[:KH].reshape(KP, FU)),
            "ufa": np.ascontiguousarray(Uc[KH:2 * KH].reshape(KP, FU)),
            "ufb": np.ascontiguousarray(Uc[2 * KH:].reshape(2 * NPL, FU)),
            "w": wslab.astype(bf),
        })
    return in_maps


def kernel(x, W, b):
    from concourse.bass_utils import run_bass_kernel_spmd

    nc = _get_nc()
    in_maps = _prep_inputs(x, W, b)
    br = run_bass_kernel_spmd(nc, in_maps, list(range(NCORES)),
                              **_cache.get("run_kwargs", {}))
    _cache["last_run"] = br
    oc = np.stack([np.asarray(br.results[i]["oc"]) for i in range(NCORES)])
    oc = oc.reshape(NCORES * RPC, 4, O, NG, B).astype(np.float32)
    out = oc.transpose(4, 2, 0, 3, 1).reshape(B, O, NCORES * RPC, RX)
    out = out[:, :, :RY, :] + np.asarray(b, np.float32).transpose(2, 0, 1)[None]
    return np.ascontiguousarray(out)
